# revision 19
# baseline (speedup 1.0000x reference)
"""Trainium2 Bass kernel: MeshGraphNet-style GNN message passing (v2).

Strategy (8 NeuronCores, SPMD):
  - Sort edges by dst. 128-node blocks; core c owns blocks [c*BPC,(c+1)*BPC).
  - All matmul operands bf16 (PSUM f32): ~4-5x PE throughput vs fp32 and
    FWL weight loads.
  - h[src] gathered FEATURE-MAJOR in one shot via gpsimd.dma_gather
    (transpose=True, 256B bf16 rows): no per-tile indirect DMAs, no PE
    transposes.  int16 index limit (32767) handled by splitting each
    block's edges into src<32768 / src>=32768 segments, gathered from a
    base-offset view of the h table.  Uniform segment caps (CA/CB) across
    all blocks/cores keep the program SPMD.
  - h[dst] term of edge-MLP layer 1 fused: Zd = (W1d^T hblk^T) computed
    once per block node-major, then expanded per-edge inside the W1
    accumulation via the host-precomputed one-hot O_T (dloc==node).
  - scatter-mean via one-hot matmuls of the HIDDEN a2 (pre-W3) into a
    [128n,128f] PSUM accumulator C; W3 is applied once per block after
    the scatter (linear ops commute), with inv_deg folded into the C
    copy-out and b3 added as a rank-1 (b3 x mask) matmul.
  - Node-update MLP feature-major; residual added via identity-matmul
    accumulation; AllGather (bf16) of updated node states per step.
"""

import os
import numpy as np

P = 128
HALF = 32768
GATHER_MODE = os.environ.get("BASS_GATHER_MODE", "indirect1")

LAST = {}


def _ceil(a, b):
    return -(-a // b) * b


def _strips(T, w=4):
    out = []
    t0 = 0
    while t0 < T:
        k = min(w, T - t0)
        out.append((t0, k))
        t0 += k
    return out


def prep_host(inputs, n_cores=8):
    x = np.asarray(inputs["x"], np.float32)
    ea = np.asarray(inputs["edge_attr"], np.float32)
    ei = np.asarray(inputs["edge_index"], np.int32)
    N, NI = x.shape
    E, EI = ea.shape
    L = np.asarray(inputs["ne_W1"]).shape[1]
    OD = np.asarray(inputs["de_W3"]).shape[1]
    S = np.asarray(inputs["pe_W1"]).shape[0]

    NB = -(-N // P)
    NB = -(-NB // n_cores) * n_cores
    BPC = NB // n_cores
    N_pad = NB * P
    N_own = BPC * P

    src = ei[0].astype(np.int64)
    dst = ei[1].astype(np.int64)
    perm = np.argsort(dst, kind="stable")
    src_s = src[perm].astype(np.int32)
    dst_s = dst[perm].astype(np.int32)
    ea_s = ea[perm]

    deg = np.bincount(dst, minlength=N_pad).astype(np.float32)
    inv_deg = (1.0 / np.maximum(deg, 1.0)).astype(np.float32)
    mask = (deg > 0).astype(np.float32)

    block_start = np.searchsorted(dst_s, np.arange(0, N_pad + 1, P))

    # per-block segment split (src < HALF vs >=) and global uniform caps
    segs = []
    nA_max = nB_max = 0
    for b in range(NB):
        s0, s1 = int(block_start[b]), int(block_start[b + 1])
        sb = src_s[s0:s1]
        db = dst_s[s0:s1]
        eb = ea_s[s0:s1]
        lo = sb < HALF
        segs.append((sb[lo], db[lo], eb[lo], sb[~lo], db[~lo], eb[~lo]))
        nA_max = max(nA_max, int(lo.sum()))
        nB_max = max(nB_max, int((~lo).sum()))
    CA = max(_ceil(nA_max, P), P)
    CB = max(_ceil(nB_max, P), P)
    E_u = CA + CB
    T_u = E_u // P

    params = dict(N=N, NI=NI, E=E, EI=EI, L=L, OD=OD, S=S,
                  NB=NB, BPC=BPC, N_pad=N_pad, N_own=N_own,
                  CA=CA, CB=CB, E_u=E_u, T_u=T_u, n_cores=n_cores)

    import ml_dtypes

    def bf16_bits(a):
        return np.ascontiguousarray(
            np.asarray(a, np.float32).astype(ml_dtypes.bfloat16))

    wf = lambda n: np.asarray(inputs[n], np.float32)

    weights = {}
    # encoder/decoder weights
    for nm in ("ne_W1", "ne_W2", "ne_W3", "ee_W1", "ee_W2", "ee_W3",
               "de_W1", "de_W2", "de_W3"):
        weights[nm] = bf16_bits(wf(nm))
    # per-step W slices
    pe_W1 = wf("pe_W1")  # [S, 3L, L]
    pn_W1 = wf("pn_W1")  # [S, 2L, L]
    for s in range(S):
        weights[f"pe_W1d_{s}"] = bf16_bits(pe_W1[s, 0:L])
        weights[f"pe_W1s_{s}"] = bf16_bits(pe_W1[s, L:2 * L])
        weights[f"pe_W1e_{s}"] = bf16_bits(pe_W1[s, 2 * L:3 * L])
        weights[f"pn_W1h_{s}"] = bf16_bits(pn_W1[s, 0:L])
        weights[f"pn_W1a_{s}"] = bf16_bits(pn_W1[s, L:2 * L])
        weights[f"pe_W2_{s}"] = bf16_bits(wf("pe_W2")[s])
        weights[f"pe_W3_{s}"] = bf16_bits(wf("pe_W3")[s])
        weights[f"pn_W2_{s}"] = bf16_bits(wf("pn_W2")[s])
        weights[f"pn_W3_{s}"] = bf16_bits(wf("pn_W3")[s])
    # biases: column f32 for activation-bias; rows bf16 for rank-1 matmuls
    for nm in ("ne_b1", "ne_b2", "ee_b1", "ee_b2", "ee_b3", "de_b1", "de_b2"):
        weights[nm] = wf(nm).reshape(L, 1).astype(np.float32)
    weights["ne_b3"] = bf16_bits(wf("ne_b3").reshape(1, L))
    weights["de_b3"] = bf16_bits(wf("de_b3").reshape(1, OD))
    for s in range(S):
        for nm in ("pe_b1", "pe_b2", "pn_b1", "pn_b2"):
            weights[f"{nm}_{s}"] = wf(nm)[s].reshape(L, 1).astype(np.float32)
        weights[f"pe_b3_{s}"] = bf16_bits(wf("pe_b3")[s].reshape(1, L))
        weights[f"pn_b3_{s}"] = bf16_bits(wf("pn_b3")[s].reshape(1, L))

    in_maps = []
    for c in range(n_cores):
        b0 = c * BPC
        eafm = np.zeros((EI, BPC * E_u), np.float32)
        OT = np.zeros((BPC * P, E_u), np.float32)
        OJ = np.zeros((BPC * P, E_u), np.float32)
        idx32 = np.zeros((BPC * P, T_u), np.int32)
        invd = np.zeros((P, BPC), np.float32)
        maskr = np.zeros((1, BPC * P), np.float32)
        for j in range(BPC):
            b = b0 + j
            sA, dA, eA, sB, dB, eB = segs[b]
            nA, nB_ = len(sA), len(sB)
            # column layout: [segA | padA | segB | padB]
            cols_src = np.zeros(E_u, np.int32)
            cols_src[:nA] = sA
            cols_src[CA:CA + nB_] = sB - HALF
            dloc = np.full(E_u, -1, np.int32)
            dloc[:nA] = dA - b * P
            dloc[CA:CA + nB_] = dB - b * P
            eac = np.zeros((E_u, EI), np.float32)
            eac[:nA] = eA
            eac[CA:CA + nB_] = eB
            eafm[:, j * E_u:(j + 1) * E_u] = eac.T
            valid = dloc >= 0
            OT[j * P:(j + 1) * P, :] = (
                dloc[None, :] == np.arange(P)[:, None]) & valid[None, :]
            # OJ: row e (partition), col t*128+n: (dloc[t*128+e] == n)
            dl2 = dloc.reshape(T_u, P)
            OJ[j * P:(j + 1) * P, :] = (
                dl2.T[:, :, None] == np.arange(P)[None, None, :]
            ).reshape(P, T_u * P)
            gsrc = cols_src.copy()
            gsrc[CA:] += HALF
            idx32[j * P:(j + 1) * P, :] = gsrc.reshape(T_u, P).T
            invd[:, j] = inv_deg[b * P:(b + 1) * P]
            maskr[0, j * P:(j + 1) * P] = mask[b * P:(b + 1) * P]
        x_own = np.zeros((NI, N_own), np.float32)
        nreal = min(N_own, max(0, N - b0 * P))
        x_own[:, :nreal] = x.T[:, b0 * P:b0 * P + nreal]

        import ml_dtypes as _mld
        otx = np.concatenate([
            np.asarray(bf16_bits(OT)),
            np.asarray(bf16_bits(OJ)),
            idx32.view(np.uint16).astype(np.uint16).view(_mld.bfloat16),
        ], axis=1)
        m = dict(weights)
        m["eafm"] = bf16_bits(eafm)
        m["otx"] = np.ascontiguousarray(otx)
        m["invd"] = np.ascontiguousarray(invd)
        m["maskr"] = bf16_bits(maskr)
        m["xfm"] = bf16_bits(x_own)
        in_maps.append(m)
    return params, in_maps


def build_program(params, debug=False):
    import concourse.bass as bass
    import concourse.bacc as bacc
    import concourse.mybir as mybir
    import concourse.tile as tile
    from concourse.bass import ds, ts
    from concourse.masks import make_identity
    from contextlib import ExitStack

    f32 = mybir.dt.float32
    bf16 = mybir.dt.bfloat16
    i16 = mybir.dt.int16
    i32 = mybir.dt.int32
    Relu = mybir.ActivationFunctionType.Relu
    AO = mybir.AluOpType

    NI, EI, L, OD, S = (params[k] for k in ("NI", "EI", "L", "OD", "S"))
    BPC, N_pad, N_own = (params[k] for k in ("BPC", "N_pad", "N_own"))
    CA, CB, E_u, T_u = (params[k] for k in ("CA", "CB", "E_u", "T_u"))
    n_cores = params["n_cores"]
    strips = _strips(T_u)

    nc = bacc.Bacc(None, target_bir_lowering=False, debug=debug)

    def par(name, shape, dtype=f32, out=False):
        return nc.declare_dram_parameter(name, list(shape), dtype, isOutput=out)

    eafm_d = par("eafm", [EI, BPC * E_u], bf16)
    otx_d = par("otx", [BPC * P, 2 * E_u + 2 * T_u], bf16)
    invd_d = par("invd", [P, BPC])
    maskr_d = par("maskr", [1, BPC * P], bf16)
    xfm_d = par("xfm", [NI, N_own], bf16)

    w_d = {}
    wshapes = {"ne_W1": [NI, L], "ne_W2": [L, L], "ne_W3": [L, L],
               "ee_W1": [EI, L], "ee_W2": [L, L], "ee_W3": [L, L],
               "de_W1": [L, L], "de_W2": [L, L], "de_W3": [L, OD]}
    for s in range(S):
        for nm in ("pe_W1d", "pe_W1s", "pe_W1e", "pe_W2", "pe_W3",
                   "pn_W1h", "pn_W1a", "pn_W2", "pn_W3"):
            wshapes[f"{nm}_{s}"] = [L, L]
    for nm, shp in wshapes.items():
        w_d[nm] = par(nm, shp, bf16)
    bshapes = {"ne_b1": [L, 1], "ne_b2": [L, 1], "ee_b1": [L, 1],
               "ee_b2": [L, 1], "ee_b3": [L, 1], "de_b1": [L, 1],
               "de_b2": [L, 1]}
    for s in range(S):
        for nm in ("pe_b1", "pe_b2", "pn_b1", "pn_b2"):
            bshapes[f"{nm}_{s}"] = [L, 1]
    for nm, shp in bshapes.items():
        w_d[nm] = par(nm, shp, f32)
    rshapes = {"ne_b3": [1, L], "de_b3": [1, OD]}
    for s in range(S):
        rshapes[f"pe_b3_{s}"] = [1, L]
        rshapes[f"pn_b3_{s}"] = [1, L]
    for nm, shp in rshapes.items():
        w_d[nm] = par(nm, shp, bf16)

    out_d = par("out", [N_own, OD], out=True)

    h_ownA = nc.dram_tensor("h_ownA", [N_own, L], bf16)
    h_ownB = nc.dram_tensor("h_ownB", [N_own, L], bf16)
    h_bufs = [nc.dram_tensor(f"h_nm{i}", [N_pad, L], bf16,
                             addr_space="Shared") for i in range(3)]
    eblk = nc.dram_tensor("eblk", [BPC * P, E_u], bf16)

    with tile.TileContext(nc) as tc, ExitStack() as ctx:
        wp = ctx.enter_context(tc.tile_pool(name="wp", bufs=1))
        sbx = ctx.enter_context(tc.tile_pool(name="sbx", bufs=3))
        sbe = ctx.enter_context(tc.tile_pool(name="sbe", bufs=3))
        sbo = ctx.enter_context(tc.tile_pool(name="sbo", bufs=3))
        sbh = ctx.enter_context(tc.tile_pool(name="sbh", bufs=3))
        sbg = ctx.enter_context(tc.tile_pool(name="sbg", bufs=2))
        sbi = ctx.enter_context(tc.tile_pool(name="sbi", bufs=3))
        sbs = ctx.enter_context(tc.tile_pool(name="sbs", bufs=4))
        sbm = ctx.enter_context(tc.tile_pool(name="sbm", bufs=8))
        ps_b = ctx.enter_context(tc.tile_pool(name="ps_b", bufs=3, space="PSUM"))
        ps_s = ctx.enter_context(tc.tile_pool(name="ps_s", bufs=2, space="PSUM"))
        ps_t = ctx.enter_context(tc.tile_pool(name="ps_t", bufs=2, space="PSUM"))
        ps_c = ctx.enter_context(tc.tile_pool(name="ps_c", bufs=1, space="PSUM"))

        identity_f = wp.tile([P, P], f32, tag="identity_f")
        make_identity(nc, identity_f[:])
        identity = wp.tile([P, P], bf16, tag="identity")
        nc.vector.tensor_copy(identity[:], identity_f[:])
        ones_row = wp.tile([1, P], bf16, tag="ones_row")
        nc.vector.memset(ones_row[:], 1.0)

        W = {}

        def load(nm, dt, shape=None):
            shp = shape or wshapes.get(nm) or bshapes.get(nm) or rshapes[nm]
            t = wp.tile(list(shp), dt, tag=f"w_{nm}")
            nc.sync.dma_start(out=t[:], in_=w_d[nm][:, :])
            W[nm] = t

        for nm in wshapes:
            load(nm, bf16)
        for nm in bshapes:
            load(nm, f32)
        for nm in rshapes:
            load(nm, bf16)

        invd_t = wp.tile([P, BPC], f32, tag="invd_t")
        nc.sync.dma_start(out=invd_t[:], in_=invd_d[:, :])
        maskr_t = wp.tile([1, BPC * P], bf16, tag="maskr_t")
        nc.sync.dma_start(out=maskr_t[:], in_=maskr_d[:, :])

        mm = nc.tensor.matmul

        # ---- node encoder (own shard), 128-node chunks ----
        with tc.For_i(0, BPC, 1) as cch:
            x_t = sbx.tile([NI, P], bf16, tag="x_t")
            nc.sync.dma_start(out=x_t[:], in_=xfm_d[:, ts(cch, P)])
            p1 = ps_s.tile([P, P], f32, tag="mm_small")
            mm(out=p1[:], lhsT=W["ne_W1"][:], rhs=x_t[:], start=True, stop=True)
            a1 = sbm.tile([P, P], bf16, tag="enc_a1")
            nc.scalar.activation(out=a1[:], in_=p1[:], func=Relu,
                                 bias=W["ne_b1"][:, :1])
            p2 = ps_s.tile([P, P], f32, tag="mm_small")
            mm(out=p2[:], lhsT=W["ne_W2"][:], rhs=a1[:], start=True, stop=True)
            a2 = sbm.tile([P, P], bf16, tag="enc_a2")
            nc.scalar.activation(out=a2[:], in_=p2[:], func=Relu,
                                 bias=W["ne_b2"][:, :1])
            p3 = ps_s.tile([P, L], f32, tag="mm_small")
            mm(out=p3[:], lhsT=a2[:], rhs=W["ne_W3"][:],
               start=True, stop=False)
            mm(out=p3[:], lhsT=ones_row[:], rhs=W["ne_b3"][:],
               start=False, stop=True)
            h_sb = sbm.tile([P, L], bf16, tag="enc_h")
            nc.vector.tensor_copy(h_sb[:], p3[:])
            nc.gpsimd.dma_start(out=h_ownA[ts(cch, P), :], in_=h_sb[:])

        # ---- edge encoder ----
        with tc.For_i(0, BPC, 1) as b:
            ea_t = sbe.tile([EI, E_u], bf16, tag="ea_t")
            nc.scalar.dma_start(out=ea_t[:], in_=eafm_d[:, ts(b, E_u)])
            e_all = sbe.tile([P, E_u], bf16, tag="e_all")
            for (t0, k) in strips:
                w = k * P
                cs = slice(t0 * P, t0 * P + w)
                p1 = ps_b.tile([P, 512], f32, tag="mm_big")
                mm(out=p1[:, :w], lhsT=W["ee_W1"][:], rhs=ea_t[:, cs],
                   start=True, stop=True)
                a1 = sbs.tile([P, 512], bf16, tag="ee_a1")
                nc.scalar.activation(out=a1[:, :w], in_=p1[:, :w], func=Relu,
                                     bias=W["ee_b1"][:, :1])
                p2 = ps_b.tile([P, 512], f32, tag="mm_big")
                mm(out=p2[:, :w], lhsT=W["ee_W2"][:], rhs=a1[:, :w],
                   start=True, stop=True)
                a2 = sbs.tile([P, 512], bf16, tag="ee_a2")
                nc.scalar.activation(out=a2[:, :w], in_=p2[:, :w], func=Relu,
                                     bias=W["ee_b2"][:, :1])
                p3 = ps_b.tile([P, 512], f32, tag="mm_big")
                mm(out=p3[:, :w], lhsT=W["ee_W3"][:], rhs=a2[:, :w],
                   start=True, stop=True)
                nc.vector.tensor_scalar_add(e_all[:, cs], p3[:, :w],
                                            W["ee_b3"][:, :1])
            nc.gpsimd.dma_start(out=eblk[ts(b, P), :], in_=e_all[:])

        nc.gpsimd.collective_compute(
            "AllGather", mybir.AluOpType.bypass,
            replica_groups=[list(range(n_cores))],
            ins=[h_ownA[:, :]], outs=[h_bufs[0][:, :]])

        # ---- message-passing steps ----
        for s in range(S):
            h_r = h_bufs[s % 3]
            h_w = h_bufs[(s + 1) % 3]
            ho_r = h_ownA if s % 2 == 0 else h_ownB
            ho_w = h_ownB if s % 2 == 0 else h_ownA
            eng_a = nc.sync if s % 2 == 0 else nc.scalar
            eng_b = nc.scalar if s % 2 == 0 else nc.sync
            with tc.For_i(0, BPC, 1) as b:
                eb_t = sbe.tile([P, E_u], bf16, tag="eb_t")
                nc.sync.dma_start(out=eb_t[:], in_=eblk[ts(b, P), :])
                otx_t = sbo.tile([P, 2 * E_u + 2 * T_u], bf16, tag="otx_t")
                nc.sync.dma_start(out=otx_t[:], in_=otx_d[ts(b, P), :])
                hblk = sbm.tile([P, L], bf16, tag="hblk")
                nc.gpsimd.dma_start(out=hblk[:], in_=ho_r[ts(b, P), :])

                hs_t = sbh.tile([P, 1, E_u], bf16, tag="hs_t")
                g_nm = sbg.tile([P, T_u * L], bf16, tag="g_nm1")
                nc.gpsimd.indirect_dma_start(
                    out=g_nm[:], out_offset=None, in_=h_r[:, :],
                    in_offset=bass.IndirectOffsetOnAxis(
                        ap=otx_t[:, 2 * E_u:2 * E_u + 2 * T_u].bitcast(i32),
                        axis=0))
                for t in range(T_u):
                    gtp = ps_t.tile([P, P], bf16, tag="mm_tr")
                    nc.tensor.transpose(out=gtp[:],
                                        in_=g_nm[:, t * L:(t + 1) * L],
                                        identity=identity[:])
                    nc.scalar.activation(
                        out=hs_t[:, 0, t * P:(t + 1) * P], in_=gtp[:],
                        func=mybir.ActivationFunctionType.Copy, bias=0.0)

                # hblk_fm = hblk^T ; Zd = hblk @ W1d (node-major)
                trp = ps_t.tile([P, P], bf16, tag="mm_tr")
                nc.tensor.transpose(out=trp[:], in_=hblk[:],
                                    identity=identity[:])
                hblk_fm = sbm.tile([P, P], bf16, tag="hblk_fm")
                nc.vector.tensor_copy(hblk_fm[:], trp[:])
                zdp = ps_s.tile([P, P], f32, tag="mm_small")
                mm(out=zdp[:], lhsT=hblk_fm[:], rhs=W[f"pe_W1d_{s}"][:],
                   start=True, stop=True)
                zd = sbm.tile([P, P], bf16, tag="zd")
                nc.vector.tensor_copy(zd[:], zdp[:])

                # edge MLP + scatter of a2 into C
                cp = ps_c.tile([P, P], f32, tag="c_acc")
                first = True
                for (t0, k) in strips:
                    w = k * P
                    cs = slice(t0 * P, t0 * P + w)
                    p1 = ps_b.tile([P, 512], f32, tag="mm_big")
                    mm(out=p1[:, :w], lhsT=zd[:], rhs=otx_t[:, cs],
                       start=True, stop=False)
                    mm(out=p1[:, :w], lhsT=W[f"pe_W1s_{s}"][:],
                       rhs=hs_t[:, 0, cs], start=False, stop=False)
                    mm(out=p1[:, :w], lhsT=W[f"pe_W1e_{s}"][:],
                       rhs=eb_t[:, cs], start=False, stop=True)
                    a1 = sbs.tile([P, 512], bf16, tag="pe_a1")
                    nc.scalar.activation(out=a1[:, :w], in_=p1[:, :w],
                                         func=Relu, bias=W[f"pe_b1_{s}"][:, :1])
                    p2 = ps_b.tile([P, 512], f32, tag="mm_big")
                    mm(out=p2[:, :w], lhsT=W[f"pe_W2_{s}"][:], rhs=a1[:, :w],
                       start=True, stop=True)
                    a2 = sbs.tile([P, 512], bf16, tag="pe_a2")
                    nc.vector.tensor_scalar(
                        out=a2[:, :w], in0=p2[:, :w],
                        scalar1=W[f"pe_b2_{s}"][:, :1], scalar2=0.0,
                        op0=AO.add, op1=AO.max)
                    for j in range(k):
                        t = t0 + j
                        tp = ps_t.tile([P, P], bf16, tag="mm_tr")
                        nc.tensor.transpose(out=tp[:],
                                            in_=a2[:, j * P:(j + 1) * P],
                                            identity=identity[:])
                        a2e = sbm.tile([P, P], bf16, tag="a2e")
                        nc.vector.tensor_copy(a2e[:], tp[:])
                        mm(out=cp[:],
                           lhsT=otx_t[:, E_u + t * P:E_u + (t + 1) * P],
                           rhs=a2e[:], start=first, stop=(t == T_u - 1))
                        first = False
                # C -> inv_deg scale -> transpose -> W3 + b3 x mask
                c_sb = sbm.tile([P, P], bf16, tag="c_sb")
                nc.vector.tensor_scalar(out=c_sb[:], in0=cp[:],
                                        scalar1=invd_t[:, ds(b, 1)],
                                        scalar2=None, op0=AO.mult)
                ctp = ps_t.tile([P, P], bf16, tag="mm_tr")
                nc.tensor.transpose(out=ctp[:], in_=c_sb[:],
                                    identity=identity[:])
                c_fm = sbm.tile([P, P], bf16, tag="c_fm")
                nc.vector.tensor_copy(c_fm[:], ctp[:])
                aggp = ps_s.tile([P, P], f32, tag="mm_small")
                mm(out=aggp[:], lhsT=W[f"pe_W3_{s}"][:], rhs=c_fm[:],
                   start=True, stop=False)
                mm(out=aggp[:], lhsT=W[f"pe_b3_{s}"][:],
                   rhs=maskr_t[:, ts(b, P)], start=False, stop=True)
                agg_fm = sbm.tile([P, P], bf16, tag="agg_fm")
                nc.vector.tensor_copy(agg_fm[:], aggp[:])

                # node-update MLP (feature-major)
                n1p = ps_s.tile([P, P], f32, tag="mm_small")
                mm(out=n1p[:], lhsT=W[f"pn_W1h_{s}"][:], rhs=hblk_fm[:],
                   start=True, stop=False)
                mm(out=n1p[:], lhsT=W[f"pn_W1a_{s}"][:], rhs=agg_fm[:],
                   start=False, stop=True)
                n1 = sbm.tile([P, P], bf16, tag="n1")
                nc.scalar.activation(out=n1[:], in_=n1p[:], func=Relu,
                                     bias=W[f"pn_b1_{s}"][:, :1])
                n2p = ps_s.tile([P, P], f32, tag="mm_small")
                mm(out=n2p[:], lhsT=W[f"pn_W2_{s}"][:], rhs=n1[:],
                   start=True, stop=True)
                n2 = sbm.tile([P, P], bf16, tag="n2")
                nc.scalar.activation(out=n2[:], in_=n2p[:], func=Relu,
                                     bias=W[f"pn_b2_{s}"][:, :1])
                n3p = ps_s.tile([P, P], f32, tag="mm_small")
                mm(out=n3p[:], lhsT=n2[:], rhs=W[f"pn_W3_{s}"][:],
                   start=True, stop=False)
                mm(out=n3p[:], lhsT=ones_row[:], rhs=W[f"pn_b3_{s}"][:],
                   start=False, stop=False)
                mm(out=n3p[:], lhsT=identity[:], rhs=hblk[:],
                   start=False, stop=True)
                hnew = sbm.tile([P, L], bf16, tag="hnew")
                nc.vector.tensor_copy(hnew[:], n3p[:])
                nc.scalar.dma_start(out=ho_w[ts(b, P), :], in_=hnew[:])
            if s < S - 1:
                nc.gpsimd.collective_compute(
                    "AllGather", mybir.AluOpType.bypass,
                    replica_groups=[list(range(n_cores))],
                    ins=[ho_w[:, :]], outs=[h_w[:, :]])

        # ---- decoder ----
        h_fin = h_ownB if S % 2 == 1 else h_ownA
        with tc.For_i(0, BPC, 1) as b:
            hblk = sbm.tile([P, L], bf16, tag="dec_hblk")
            nc.scalar.dma_start(out=hblk[:], in_=h_fin[ts(b, P), :])
            trp = ps_t.tile([P, P], bf16, tag="mm_tr")
            nc.tensor.transpose(out=trp[:], in_=hblk[:], identity=identity[:])
            hfm = sbm.tile([P, P], bf16, tag="dec_hfm")
            nc.vector.tensor_copy(hfm[:], trp[:])
            d1p = ps_s.tile([P, P], f32, tag="mm_small")
            mm(out=d1p[:], lhsT=W["de_W1"][:], rhs=hfm[:], start=True, stop=True)
            d1 = sbm.tile([P, P], bf16, tag="d1")
            nc.scalar.activation(out=d1[:], in_=d1p[:], func=Relu,
                                 bias=W["de_b1"][:, :1])
            d2p = ps_s.tile([P, P], f32, tag="mm_small")
            mm(out=d2p[:], lhsT=W["de_W2"][:], rhs=d1[:], start=True, stop=True)
            d2 = sbm.tile([P, P], bf16, tag="d2")
            nc.scalar.activation(out=d2[:], in_=d2p[:], func=Relu,
                                 bias=W["de_b2"][:, :1])
            dp = ps_s.tile([P, OD], f32, tag="mm_small")
            mm(out=dp[:], lhsT=d2[:], rhs=W["de_W3"][:], start=True, stop=False)
            mm(out=dp[:], lhsT=ones_row[:], rhs=W["de_b3"][:],
               start=False, stop=True)
            osb = sbm.tile([P, OD], f32, tag="osb")
            nc.vector.tensor_copy(osb[:], dp[:])
            nc.scalar.dma_start(out=out_d[ts(b, P), :], in_=osb[:])

    nc.finalize()
    return nc


def _ensure_ntff_hook():
    """Register the axon NTFF profiling hook if the image lacks
    antenv.axon_hooks (replicates trn_boot's ctypes wiring)."""
    import sys
    import types
    try:
        import antenv.axon_hooks  # noqa: F401
        return
    except ImportError:
        pass
    import contextlib
    import ctypes
    import antenv

    m = types.ModuleType("antenv.axon_hooks")
    state = {"hook": None, "tried": False}

    def set_axon_ntff_profile_hook(hook):
        state["hook"] = hook

    def _make_hook(so_path="/opt/axon/libaxon_pjrt.so"):
        lib = ctypes.CDLL(so_path)
        if not hasattr(lib, "axon_start_nrt_profile"):
            return None
        lib.axon_start_nrt_profile.argtypes = [
            ctypes.POINTER(ctypes.c_int64), ctypes.c_size_t]
        lib.axon_start_nrt_profile.restype = ctypes.c_int64
        lib.axon_stop_nrt_profile.argtypes = [ctypes.c_char_p]
        lib.axon_stop_nrt_profile.restype = ctypes.c_int64

        @contextlib.contextmanager
        def _hook(output_dir, device_ids):
            import jax
            jax.devices()
            if device_ids:
                ids = (ctypes.c_int64 * len(device_ids))(*device_ids)
                rc = lib.axon_start_nrt_profile(ids, len(device_ids))
            else:
                rc = lib.axon_start_nrt_profile(None, 0)
            if rc != 0:
                raise RuntimeError(f"axon_start_nrt_profile rc={rc}")
            try:
                yield
            finally:
                n = lib.axon_stop_nrt_profile(str(output_dir).encode())
                print(f"ntff profile: {n} file(s) written to {output_dir}")

        return _hook

    def get_axon_ntff_profile_hook():
        if state["hook"] is None and not state["tried"]:
            state["tried"] = True
            try:
                state["hook"] = _make_hook()
            except OSError:
                state["hook"] = None
        return state["hook"]

    m.set_axon_ntff_profile_hook = set_axon_ntff_profile_hook
    m.get_axon_ntff_profile_hook = get_axon_ntff_profile_hook
    sys.modules["antenv.axon_hooks"] = m
    antenv.axon_hooks = m


def kernel(**inputs):
    n_cores = 8
    params, in_maps = prep_host(inputs, n_cores)
    nc = build_program(params, debug=False)

    from concourse.bass_utils import run_bass_kernel_spmd
    import time
    trace = bool(int(os.environ.get("KERNEL_TRACE", "0")))
    if trace:
        try:
            _ensure_ntff_hook()
        except Exception:
            pass
    t0 = time.time()
    try:
        res = run_bass_kernel_spmd(nc, in_maps, list(range(n_cores)),
                                   trace=trace)
    except ModuleNotFoundError:
        res = run_bass_kernel_spmd(nc, in_maps, list(range(n_cores)),
                                   trace=False)
    LAST["wall_s"] = time.time() - t0
    LAST["exec_time_ns"] = getattr(res, "exec_time_ns", None)
    LAST["profile_json"] = getattr(res, "profile_json", None)
    LAST["params"] = params
    out = np.concatenate([r["out"] for r in res.results], axis=0)
    return np.ascontiguousarray(out[:params["N"]].astype(np.float32))


# revision 38
# speedup vs baseline: 1.4491x; 1.4491x over previous
"""Trainium2 Bass kernel: MeshGraphNet-style GNN message passing (v2).

Strategy (8 NeuronCores, SPMD):
  - Sort edges by dst. 128-node blocks; core c owns blocks [c*BPC,(c+1)*BPC).
  - All matmul operands bf16 (PSUM f32): ~4-5x PE throughput vs fp32 and
    FWL weight loads.
  - h[src] gathered FEATURE-MAJOR in one shot via gpsimd.dma_gather
    (transpose=True, 256B bf16 rows): no per-tile indirect DMAs, no PE
    transposes.  int16 index limit (32767) handled by splitting each
    block's edges into src<32768 / src>=32768 segments, gathered from a
    base-offset view of the h table.  Uniform segment caps (CA/CB) across
    all blocks/cores keep the program SPMD.
  - h[dst] term of edge-MLP layer 1 fused: Zd = (W1d^T hblk^T) computed
    once per block node-major, then expanded per-edge inside the W1
    accumulation via the host-precomputed one-hot O_T (dloc==node).
  - scatter-mean via one-hot matmuls of the HIDDEN a2 (pre-W3) into a
    [128n,128f] PSUM accumulator C; W3 is applied once per block after
    the scatter (linear ops commute), with inv_deg folded into the C
    copy-out and b3 added as a rank-1 (b3 x mask) matmul.
  - Node-update MLP feature-major; residual added via identity-matmul
    accumulation; AllGather (bf16) of updated node states per step.
"""

import os
import numpy as np

P = 128
HALF = 32768
GATHER_MODE = os.environ.get("BASS_GATHER_MODE", "indirect1")

LAST = {}


def _ceil(a, b):
    return -(-a // b) * b


def _strips(T, w=4):
    out = []
    t0 = 0
    while t0 < T:
        k = min(w, T - t0)
        out.append((t0, k))
        t0 += k
    return out


def prep_host(inputs, n_cores=8):
    x = np.asarray(inputs["x"], np.float32)
    ea = np.asarray(inputs["edge_attr"], np.float32)
    ei = np.asarray(inputs["edge_index"], np.int32)
    N, NI = x.shape
    E, EI = ea.shape
    L = np.asarray(inputs["ne_W1"]).shape[1]
    OD = np.asarray(inputs["de_W3"]).shape[1]
    S = np.asarray(inputs["pe_W1"]).shape[0]

    NB = -(-N // P)
    NB = -(-NB // n_cores) * n_cores
    BPC = NB // n_cores
    N_pad = NB * P
    N_own = BPC * P

    src = ei[0].astype(np.int64)
    dst = ei[1].astype(np.int64)
    perm = np.argsort(dst, kind="stable")
    src_s = src[perm].astype(np.int32)
    dst_s = dst[perm].astype(np.int32)
    ea_s = ea[perm]

    deg = np.bincount(dst, minlength=N_pad).astype(np.float32)
    inv_deg = (1.0 / np.maximum(deg, 1.0)).astype(np.float32)
    mask = (deg > 0).astype(np.float32)

    block_start = np.searchsorted(dst_s, np.arange(0, N_pad + 1, P))

    # per-block segment split (src < HALF vs >=) and global uniform caps
    segs = []
    nA_max = nB_max = 0
    for b in range(NB):
        s0, s1 = int(block_start[b]), int(block_start[b + 1])
        sb = src_s[s0:s1]
        db = dst_s[s0:s1]
        eb = ea_s[s0:s1]
        lo = sb < HALF
        segs.append((sb[lo], db[lo], eb[lo], sb[~lo], db[~lo], eb[~lo]))
        nA_max = max(nA_max, int(lo.sum()))
        nB_max = max(nB_max, int((~lo).sum()))
    CA = max(_ceil(nA_max, P), P)
    CB = max(_ceil(nB_max, P), P)
    E_u = CA + CB
    T_u = E_u // P

    params = dict(N=N, NI=NI, E=E, EI=EI, L=L, OD=OD, S=S,
                  NB=NB, BPC=BPC, N_pad=N_pad, N_own=N_own,
                  CA=CA, CB=CB, E_u=E_u, T_u=T_u, n_cores=n_cores)

    import ml_dtypes

    def bf16_bits(a):
        return np.ascontiguousarray(
            np.asarray(a, np.float32).astype(ml_dtypes.bfloat16))

    wf = lambda n: np.asarray(inputs[n], np.float32)

    weights = {}
    # encoder/decoder weights
    for nm in ("ne_W1", "ne_W2", "ne_W3", "ee_W1", "ee_W2", "ee_W3",
               "de_W1", "de_W2", "de_W3"):
        weights[nm] = bf16_bits(wf(nm))
    # per-step W slices
    pe_W1 = wf("pe_W1")  # [S, 3L, L]
    pn_W1 = wf("pn_W1")  # [S, 2L, L]
    for s in range(S):
        weights[f"pe_W1d_{s}"] = bf16_bits(pe_W1[s, 0:L])
        weights[f"pe_W1s_{s}"] = bf16_bits(pe_W1[s, L:2 * L])
        weights[f"pe_W1e_{s}"] = bf16_bits(pe_W1[s, 2 * L:3 * L])
        weights[f"pn_W1h_{s}"] = bf16_bits(pn_W1[s, 0:L])
        weights[f"pn_W1a_{s}"] = bf16_bits(pn_W1[s, L:2 * L])
        weights[f"pe_W2_{s}"] = bf16_bits(wf("pe_W2")[s])
        weights[f"pe_W3_{s}"] = bf16_bits(wf("pe_W3")[s])
        weights[f"pn_W2_{s}"] = bf16_bits(wf("pn_W2")[s])
        weights[f"pn_W3_{s}"] = bf16_bits(wf("pn_W3")[s])
    # biases: column f32 for activation-bias; rows bf16 for rank-1 matmuls
    for nm in ("ne_b1", "ne_b2", "ee_b1", "ee_b2", "ee_b3", "de_b1", "de_b2"):
        weights[nm] = wf(nm).reshape(L, 1).astype(np.float32)
    weights["ne_b3"] = bf16_bits(wf("ne_b3").reshape(1, L))
    weights["de_b3"] = bf16_bits(wf("de_b3").reshape(1, OD))
    for s in range(S):
        for nm in ("pe_b1", "pe_b2", "pn_b1", "pn_b2"):
            weights[f"{nm}_{s}"] = wf(nm)[s].reshape(L, 1).astype(np.float32)
        weights[f"pe_b3_{s}"] = bf16_bits(wf("pe_b3")[s].reshape(1, L))
        weights[f"pn_b3_{s}"] = bf16_bits(wf("pn_b3")[s].reshape(1, L))

    in_maps = []
    for c in range(n_cores):
        b0 = c * BPC
        eafm = np.zeros((EI, BPC * E_u), np.float32)
        OT = np.zeros((BPC * P, E_u), np.float32)
        OJ = np.zeros((BPC * P, E_u), np.float32)
        idx32 = np.zeros((BPC * P, T_u + 1), np.int32)
        idx16 = np.zeros((BPC * P, E_u // 16), np.int16)
        invd = np.zeros((P, BPC), np.float32)
        maskr = np.zeros((1, BPC * P), np.float32)
        for j in range(BPC):
            b = b0 + j
            sA, dA, eA, sB, dB, eB = segs[b]
            nA, nB_ = len(sA), len(sB)
            # column layout: [segA | padA | segB | padB]
            cols_src = np.zeros(E_u, np.int32)
            cols_src[:nA] = sA
            cols_src[CA:CA + nB_] = sB - HALF
            dloc = np.full(E_u, -1, np.int32)
            dloc[:nA] = dA - b * P
            dloc[CA:CA + nB_] = dB - b * P
            eac = np.zeros((E_u, EI), np.float32)
            eac[:nA] = eA
            eac[CA:CA + nB_] = eB
            eafm[:, j * E_u:(j + 1) * E_u] = eac.T
            valid = dloc >= 0
            OT[j * P:(j + 1) * P, :] = (
                dloc[None, :] == np.arange(P)[:, None]) & valid[None, :]
            # OJ: row e (partition), col t*128+n: (dloc[t*128+e] == n)
            dl2 = dloc.reshape(T_u, P)
            OJ[j * P:(j + 1) * P, :] = (
                dl2.T[:, :, None] == np.arange(P)[None, None, :]
            ).reshape(P, T_u * P)
            gsrc = cols_src.copy()
            gsrc[CA:] += HALF
            idx32[j * P:(j + 1) * P, :T_u] = gsrc.reshape(T_u, P).T
            idx32[j * P:(j + 1) * P, T_u] = np.arange(b * P, (b + 1) * P)
            ii = np.arange(E_u)
            for k in range(8):
                idx16[j * P + 16 * k + (ii % 16), ii // 16] = \
                    cols_src.astype(np.int16)
            invd[:, j] = inv_deg[b * P:(b + 1) * P]
            maskr[0, j * P:(j + 1) * P] = mask[b * P:(b + 1) * P]
        x_own = np.zeros((NI, N_own), np.float32)
        nreal = min(N_own, max(0, N - b0 * P))
        x_own[:, :nreal] = x.T[:, b0 * P:b0 * P + nreal]

        import ml_dtypes as _mld
        otx = np.concatenate([
            np.asarray(bf16_bits(OT)),
            np.asarray(bf16_bits(OJ)),
            idx32.view(np.uint16).astype(np.uint16).view(_mld.bfloat16),
        ], axis=1)
        m = dict(weights)
        m["eafm"] = bf16_bits(eafm)
        m["otx"] = np.ascontiguousarray(otx)
        m["invd"] = np.ascontiguousarray(invd)
        m["maskr"] = bf16_bits(maskr)
        m["xfm"] = bf16_bits(x_own)
        in_maps.append(m)
    return params, in_maps


def build_program(params, debug=False):
    import concourse.bass as bass
    import concourse.bacc as bacc
    import concourse.mybir as mybir
    import concourse.tile as tile
    from concourse.bass import ds, ts
    from concourse.masks import make_identity
    from contextlib import ExitStack

    f32 = mybir.dt.float32
    bf16 = mybir.dt.bfloat16
    i16 = mybir.dt.int16
    i32 = mybir.dt.int32
    Relu = mybir.ActivationFunctionType.Relu
    AO = mybir.AluOpType

    NI, EI, L, OD, S = (params[k] for k in ("NI", "EI", "L", "OD", "S"))
    BPC, N_pad, N_own = (params[k] for k in ("BPC", "N_pad", "N_own"))
    CA, CB, E_u, T_u = (params[k] for k in ("CA", "CB", "E_u", "T_u"))
    n_cores = params["n_cores"]
    strips = _strips(T_u)

    nc = bacc.Bacc(None, target_bir_lowering=False, debug=debug)

    def par(name, shape, dtype=f32, out=False):
        return nc.declare_dram_parameter(name, list(shape), dtype, isOutput=out)

    eafm_d = par("eafm", [EI, BPC * E_u], bf16)
    otx_d = par("otx", [BPC * P, 2 * E_u + 2 * (T_u + 1)], bf16)
    invd_d = par("invd", [P, BPC])
    maskr_d = par("maskr", [1, BPC * P], bf16)
    xfm_d = par("xfm", [NI, N_own], bf16)

    w_d = {}
    wshapes = {"ne_W1": [NI, L], "ne_W2": [L, L], "ne_W3": [L, L],
               "ee_W1": [EI, L], "ee_W2": [L, L], "ee_W3": [L, L],
               "de_W1": [L, L], "de_W2": [L, L], "de_W3": [L, OD]}
    for s in range(S):
        for nm in ("pe_W1d", "pe_W1s", "pe_W1e", "pe_W2", "pe_W3",
                   "pn_W1h", "pn_W1a", "pn_W2", "pn_W3"):
            wshapes[f"{nm}_{s}"] = [L, L]
    for nm, shp in wshapes.items():
        w_d[nm] = par(nm, shp, bf16)
    bshapes = {"ne_b1": [L, 1], "ne_b2": [L, 1], "ee_b1": [L, 1],
               "ee_b2": [L, 1], "ee_b3": [L, 1], "de_b1": [L, 1],
               "de_b2": [L, 1]}
    for s in range(S):
        for nm in ("pe_b1", "pe_b2", "pn_b1", "pn_b2"):
            bshapes[f"{nm}_{s}"] = [L, 1]
    for nm, shp in bshapes.items():
        w_d[nm] = par(nm, shp, f32)
    rshapes = {"ne_b3": [1, L], "de_b3": [1, OD]}
    for s in range(S):
        rshapes[f"pe_b3_{s}"] = [1, L]
        rshapes[f"pn_b3_{s}"] = [1, L]
    for nm, shp in rshapes.items():
        w_d[nm] = par(nm, shp, bf16)

    out_d = par("out", [N_own, OD], out=True)

    h_ownA = nc.dram_tensor("h_ownA", [N_own, L], bf16)
    h_ownB = nc.dram_tensor("h_ownB", [N_own, L], bf16)
    h_bufs = [nc.dram_tensor(f"h_nm{i}", [N_pad, L], bf16,
                             addr_space="Shared") for i in range(3)]
    eblk = nc.dram_tensor("eblk", [BPC * P, E_u], bf16)

    with tile.TileContext(nc) as tc, ExitStack() as ctx:
        wp = ctx.enter_context(tc.tile_pool(name="wp", bufs=1))
        sbx = ctx.enter_context(tc.tile_pool(name="sbx", bufs=3))
        sbe = ctx.enter_context(tc.tile_pool(name="sbe", bufs=3))
        sbo = ctx.enter_context(tc.tile_pool(name="sbo", bufs=2))
        sbh = ctx.enter_context(tc.tile_pool(name="sbh", bufs=2))
        sbg = ctx.enter_context(tc.tile_pool(name="sbg", bufs=2))
        sba = ctx.enter_context(tc.tile_pool(name="sba", bufs=12))
        sbi = ctx.enter_context(tc.tile_pool(name="sbi", bufs=3))
        sbs = ctx.enter_context(tc.tile_pool(name="sbs", bufs=3))
        sbm = ctx.enter_context(tc.tile_pool(name="sbm", bufs=3))
        ps_b = ctx.enter_context(tc.tile_pool(name="ps_b", bufs=2, space="PSUM"))
        ps_s = ctx.enter_context(tc.tile_pool(name="ps_s", bufs=2, space="PSUM"))
        ps_t = ctx.enter_context(tc.tile_pool(name="ps_t", bufs=2, space="PSUM"))
        ps_c = ctx.enter_context(tc.tile_pool(name="ps_c", bufs=2, space="PSUM"))

        identity_f = wp.tile([P, P], f32, tag="identity_f")
        make_identity(nc, identity_f[:])
        identity = wp.tile([P, P], bf16, tag="identity")
        nc.vector.tensor_copy(identity[:], identity_f[:])
        ones_row = wp.tile([1, P], bf16, tag="ones_row")
        nc.vector.memset(ones_row[:], 1.0)

        W = {}

        def load(nm, dt, shape=None):
            shp = shape or wshapes.get(nm) or bshapes.get(nm) or rshapes[nm]
            t = wp.tile(list(shp), dt, tag=f"w_{nm}")
            nc.sync.dma_start(out=t[:], in_=w_d[nm][:, :])
            W[nm] = t

        for nm in wshapes:
            load(nm, bf16)
        for nm in bshapes:
            load(nm, f32)
        for nm in rshapes:
            load(nm, bf16)

        invd_t = wp.tile([P, BPC], f32, tag="invd_t")
        nc.sync.dma_start(out=invd_t[:], in_=invd_d[:, :])
        maskr_t = wp.tile([1, BPC * P], bf16, tag="maskr_t")
        nc.sync.dma_start(out=maskr_t[:], in_=maskr_d[:, :])

        mm = nc.tensor.matmul

        # ---- node encoder (own shard), 128-node chunks ----
        with tc.For_i(0, BPC, 1) as cch:
            x_t = sbx.tile([NI, P], bf16, tag="x_t")
            nc.sync.dma_start(out=x_t[:], in_=xfm_d[:, ts(cch, P)])
            p1 = ps_s.tile([P, P], f32, tag="mm_small")
            mm(out=p1[:], lhsT=W["ne_W1"][:], rhs=x_t[:], start=True, stop=True)
            a1 = sbm.tile([P, P], bf16, tag="enc_a1")
            nc.scalar.activation(out=a1[:], in_=p1[:], func=Relu,
                                 bias=W["ne_b1"][:, :1])
            p2 = ps_s.tile([P, P], f32, tag="mm_small")
            mm(out=p2[:], lhsT=W["ne_W2"][:], rhs=a1[:], start=True, stop=True)
            a2 = sbm.tile([P, P], bf16, tag="enc_a2")
            nc.scalar.activation(out=a2[:], in_=p2[:], func=Relu,
                                 bias=W["ne_b2"][:, :1])
            p3 = ps_s.tile([P, L], f32, tag="mm_small")
            mm(out=p3[:], lhsT=a2[:], rhs=W["ne_W3"][:],
               start=True, stop=False)
            mm(out=p3[:], lhsT=ones_row[:], rhs=W["ne_b3"][:],
               start=False, stop=True)
            h_sb = sbm.tile([P, L], bf16, tag="enc_h")
            nc.vector.tensor_copy(h_sb[:], p3[:])
            nc.gpsimd.dma_start(out=h_ownA[ts(cch, P), :], in_=h_sb[:])

        # ---- edge encoder ----
        with tc.For_i(0, BPC, 1) as b:
            ea_t = sbe.tile([EI, E_u], bf16, tag="ea_t")
            nc.scalar.dma_start(out=ea_t[:], in_=eafm_d[:, ts(b, E_u)])
            e_all = sbe.tile([P, E_u], bf16, tag="e_all")
            for (t0, k) in strips:
                w = k * P
                cs = slice(t0 * P, t0 * P + w)
                p1 = ps_b.tile([P, 512], f32, tag="mm_big")
                mm(out=p1[:, :w], lhsT=W["ee_W1"][:], rhs=ea_t[:, cs],
                   start=True, stop=True)
                a1 = sbs.tile([P, 512], bf16, tag="ee_a1")
                nc.scalar.activation(out=a1[:, :w], in_=p1[:, :w], func=Relu,
                                     bias=W["ee_b1"][:, :1])
                p2 = ps_b.tile([P, 512], f32, tag="mm_big")
                mm(out=p2[:, :w], lhsT=W["ee_W2"][:], rhs=a1[:, :w],
                   start=True, stop=True)
                a2 = sbs.tile([P, 512], bf16, tag="ee_a2")
                nc.scalar.activation(out=a2[:, :w], in_=p2[:, :w], func=Relu,
                                     bias=W["ee_b2"][:, :1])
                p3 = ps_b.tile([P, 512], f32, tag="mm_big")
                mm(out=p3[:, :w], lhsT=W["ee_W3"][:], rhs=a2[:, :w],
                   start=True, stop=True)
                nc.vector.tensor_scalar_add(e_all[:, cs], p3[:, :w],
                                            W["ee_b3"][:, :1])
            nc.gpsimd.dma_start(out=eblk[ts(b, P), :], in_=e_all[:])

        nc.gpsimd.collective_compute(
            "AllGather", mybir.AluOpType.bypass,
            replica_groups=[list(range(n_cores))],
            ins=[h_ownA[:, :]], outs=[h_bufs[0][:, :]])

        # ---- message-passing steps (2-block interleaved) ----
        W0 = 2 * E_u + 2 * (T_u + 1)
        PAIRS = BPC // 2
        TAIL = BPC % 2

        def emit_blocks(s, h_r, eb2, ox2, bes, hblks):
            """bes: list of (lane, block_index_expr). eb2 [P, n*E_u],
            ox2 [P, n*W0]."""
            n = len(bes)
            g_nms, hfms, zds = [], [], []
            for l, be, bp in bes:
                g_nm = sbg.tile([P, (T_u + 1) * L], bf16, tag=f"g{l}")
                nc.gpsimd.indirect_dma_start(
                    out=g_nm[:], out_offset=None, in_=h_r[:, :],
                    in_offset=bass.IndirectOffsetOnAxis(
                        ap=ox2[:, l * W0 + 2 * E_u:
                               l * W0 + 2 * E_u + 2 * (T_u + 1)]
                        .bitcast(i32), axis=0))
                g_nms.append(g_nm)
            for l, be, bp in bes:
                trp = ps_t.tile([P, P], bf16, tag="mm_tr")
                nc.tensor.transpose(out=trp[:], in_=hblks[l],
                                    identity=identity[:])
                hfm = sbm.tile([P, P], bf16, tag=f"hfm{l}")
                nc.vector.tensor_copy(hfm[:], trp[:])
                hfms.append((trp, hfm))
            for l, be, bp in bes:
                zdp = ps_s.tile([P, P], f32, tag="mm_small")
                mm(out=zdp[:], lhsT=hfms[l][1][:], rhs=W[f"pe_W1d_{s}"][:],
                   start=True, stop=True)
                zd = sbm.tile([P, P], bf16, tag=f"zd{l}")
                nc.vector.tensor_copy(zd[:], zdp[:])
                zds.append(zd)
            hss = []
            for l, _, _ in bes:
                hs_l = sbh.tile([P, 1, E_u], bf16, tag=f"hs{l}")
                hss.append(hs_l)
            a2s = {l: [] for l, _, _ in bes}
            for (t0, k) in strips:
                w = k * P
                for l, be, bp in bes:
                    for j in range(k):
                        t = t0 + j
                        gtp = ps_t.tile([P, P], bf16, tag="mm_tr")
                        nc.tensor.transpose(
                            out=gtp[:], in_=g_nms[l][:, t * L:(t + 1) * L],
                            identity=identity[:])
                        nc.vector.tensor_copy(
                            hss[l][:, 0, t * P:(t + 1) * P], gtp[:])
                p1s = []
                for l, be, bp in bes:
                    cs = slice(l * E_u + t0 * P, l * E_u + t0 * P + w)
                    hcs = slice(l * W0 + t0 * P, l * W0 + t0 * P + w)
                    p1 = ps_b.tile([P, 512], f32, tag="mm_big")
                    mm(out=p1[:, :w], lhsT=zds[l][:], rhs=ox2[:, hcs],
                       start=True, stop=False)
                    mm(out=p1[:, :w], lhsT=W[f"pe_W1s_{s}"][:],
                       rhs=hss[l][:, 0, t0 * P:t0 * P + w],
                       start=False, stop=False)
                    mm(out=p1[:, :w], lhsT=W[f"pe_W1e_{s}"][:],
                       rhs=eb2[:, cs], start=False, stop=True)
                    a1 = sbs.tile([P, 512], bf16, tag="pe_a1")
                    nc.scalar.activation(out=a1[:, :w], in_=p1[:, :w],
                                         func=Relu,
                                         bias=W[f"pe_b1_{s}"][:, :1])
                    p1s.append(a1)
                for l, be, bp in bes:
                    a1 = p1s[l]
                    p2 = ps_b.tile([P, 512], f32, tag="mm_big")
                    mm(out=p2[:, :w], lhsT=W[f"pe_W2_{s}"][:], rhs=a1[:, :w],
                       start=True, stop=True)
                    a2 = sba.tile([P, 512], bf16, tag="pe_a2")
                    nc.scalar.activation(out=a2[:, :w], in_=p2[:, :w],
                                         func=Relu,
                                         bias=W[f"pe_b2_{s}"][:, :1])
                    a2s[l].append((a2, t0, k))
            cps = []
            for l, _, _ in bes:
                cp_l = ps_c.tile([P, P], f32, tag="c_acc")
                cps.append(cp_l)
            tps = {}
            for t in range(T_u):
                si = t // 4
                jj = t % 4
                for l, be, bp in bes:
                    a2, t0, k = a2s[l][si]
                    tp = ps_t.tile([P, P], bf16, tag="mm_tr")
                    nc.tensor.transpose(out=tp[:],
                                        in_=a2[:, jj * P:(jj + 1) * P],
                                        identity=identity[:])
                    a2e = sbm.tile([P, P], bf16, tag=f"a2e{l}")
                    nc.vector.tensor_copy(a2e[:], tp[:])
                    tps[l] = a2e
                for l, be, bp in bes:
                    mm(out=cps[l][:],
                       lhsT=ox2[:, l * W0 + E_u + t * P:
                                l * W0 + E_u + (t + 1) * P],
                       rhs=tps[l][:], start=(t == 0), stop=(t == T_u - 1))
            csbs, cfms, aggs = [], [], []
            for l, be, bp in bes:
                c_sb = sbm.tile([P, P], bf16, tag=f"csb{l}")
                nc.vector.tensor_scalar(out=c_sb[:], in0=cps[l][:],
                                        scalar1=invd_t[:, ds(be, 1)],
                                        scalar2=None, op0=AO.mult)
                csbs.append(c_sb)
            for l, be, bp in bes:
                ctp = ps_t.tile([P, P], bf16, tag="mm_tr")
                nc.tensor.transpose(out=ctp[:], in_=csbs[l][:],
                                    identity=identity[:])
                c_fm = sbm.tile([P, P], bf16, tag=f"cfm{l}")
                nc.vector.tensor_copy(c_fm[:], ctp[:])
                cfms.append(c_fm)
            for l, be, bp in bes:
                aggp = ps_s.tile([P, P], f32, tag="mm_small")
                mm(out=aggp[:], lhsT=W[f"pe_W3_{s}"][:], rhs=cfms[l][:],
                   start=True, stop=False)
                mm(out=aggp[:], lhsT=W[f"pe_b3_{s}"][:],
                   rhs=maskr_t[:, ds(bp, P)], start=False, stop=True)
                agg_fm = sbm.tile([P, P], bf16, tag=f"agg{l}")
                nc.vector.tensor_copy(agg_fm[:], aggp[:])
                aggs.append(agg_fm)
            n1s, n2s = [], []
            for l, be, bp in bes:
                n1p = ps_s.tile([P, P], f32, tag="mm_small")
                mm(out=n1p[:], lhsT=W[f"pn_W1h_{s}"][:], rhs=hfms[l][1][:],
                   start=True, stop=False)
                mm(out=n1p[:], lhsT=W[f"pn_W1a_{s}"][:], rhs=aggs[l][:],
                   start=False, stop=True)
                n1 = sbm.tile([P, P], bf16, tag=f"n1{l}")
                nc.scalar.activation(out=n1[:], in_=n1p[:], func=Relu,
                                     bias=W[f"pn_b1_{s}"][:, :1])
                n1s.append(n1)
            for l, be, bp in bes:
                n2p = ps_s.tile([P, P], f32, tag="mm_small")
                mm(out=n2p[:], lhsT=W[f"pn_W2_{s}"][:], rhs=n1s[l][:],
                   start=True, stop=True)
                n2 = sbm.tile([P, P], bf16, tag=f"n2{l}")
                nc.scalar.activation(out=n2[:], in_=n2p[:], func=Relu,
                                     bias=W[f"pn_b2_{s}"][:, :1])
                n2s.append(n2)
            hnew2 = sbm.tile([P, len(bes) * L], bf16, tag="hnew2")
            for l, be, bp in bes:
                n3p = ps_s.tile([P, P], f32, tag="mm_small")
                mm(out=n3p[:], lhsT=n2s[l][:], rhs=W[f"pn_W3_{s}"][:],
                   start=True, stop=False)
                mm(out=n3p[:], lhsT=ones_row[:], rhs=W[f"pn_b3_{s}"][:],
                   start=False, stop=False)
                mm(out=n3p[:], lhsT=identity[:], rhs=hblks[l],
                   start=False, stop=True)
                nc.vector.tensor_copy(hnew2[:, l * L:(l + 1) * L], n3p[:])
            return hnew2

        for s in range(S):
            h_r = h_bufs[s % 3]
            h_w = h_bufs[(s + 1) % 3]
            ho_w = h_ownB if s % 2 == 0 else h_ownA
            ho_r = h_ownA if s % 2 == 0 else h_ownB
            with tc.For_i(0, PAIRS, 1) as ip:
                ox2 = sbo.tile([P, 2 * W0], bf16, tag="ox2")
                nc.sync.dma_start(
                    out=ox2[:],
                    in_=otx_d[ds(ip * (2 * P), 2 * P), :]
                    .rearrange("(l p) e -> p l e", l=2))
                eb2 = sbe.tile([P, 2 * E_u], bf16, tag="eb2")
                nc.sync.dma_start(
                    out=eb2[:],
                    in_=eblk[ds(ip * (2 * P), 2 * P), :]
                    .rearrange("(l p) e -> p l e", l=2))
                hblk2 = sbm.tile([P, 2, L], bf16, tag="hblk2")
                nc.gpsimd.dma_start(
                    out=hblk2[:],
                    in_=ho_r[ds(ip * (2 * P), 2 * P), :]
                    .rearrange("(l p) e -> p l e", l=2))
                hnew2 = emit_blocks(
                    s, h_r, eb2, ox2,
                    [(0, ip * 2, ip * (2 * P)),
                     (1, ip * 2 + 1, ip * (2 * P) + P)],
                    [hblk2[:, 0, :], hblk2[:, 1, :]])
                nc.scalar.dma_start(
                    out=ho_w[ds(ip * (2 * P), 2 * P), :]
                    .rearrange("(l p) e -> p l e", l=2),
                    in_=hnew2[:])
            if TAIL:
                bt = BPC - 1
                eb2 = sbe.tile([P, 2 * E_u], bf16, tag="eb2")
                nc.sync.dma_start(out=eb2[:, 0:E_u],
                                  in_=eblk[ds(bt * P, P), :])
                ox2 = sbo.tile([P, 2 * W0], bf16, tag="ox2")
                nc.sync.dma_start(out=ox2[:, 0:W0],
                                  in_=otx_d[ds(bt * P, P), :])
                hblkt = sbm.tile([P, L], bf16, tag="hblkt")
                nc.gpsimd.dma_start(out=hblkt[:], in_=ho_r[ds(bt * P, P), :])
                hnew2 = emit_blocks(s, h_r, eb2, ox2, [(0, bt, bt * P)],
                                    [hblkt[:]])
                nc.scalar.dma_start(out=ho_w[ds(bt * P, P), :],
                                    in_=hnew2[:])
            if s < S - 1:
                nc.gpsimd.collective_compute(
                    "AllGather", mybir.AluOpType.bypass,
                    replica_groups=[list(range(n_cores))],
                    ins=[ho_w[:, :]], outs=[h_w[:, :]])

        # ---- decoder ----
        h_fin = h_ownB if S % 2 == 1 else h_ownA
        with tc.For_i(0, BPC, 1) as b:
            hblk = sbm.tile([P, L], bf16, tag="dec_hblk")
            nc.scalar.dma_start(out=hblk[:], in_=h_fin[ts(b, P), :])
            trp = ps_t.tile([P, P], bf16, tag="mm_tr")
            nc.tensor.transpose(out=trp[:], in_=hblk[:], identity=identity[:])
            hfm = sbm.tile([P, P], bf16, tag="dec_hfm")
            nc.vector.tensor_copy(hfm[:], trp[:])
            d1p = ps_s.tile([P, P], f32, tag="mm_small")
            mm(out=d1p[:], lhsT=W["de_W1"][:], rhs=hfm[:], start=True, stop=True)
            d1 = sbm.tile([P, P], bf16, tag="d1")
            nc.scalar.activation(out=d1[:], in_=d1p[:], func=Relu,
                                 bias=W["de_b1"][:, :1])
            d2p = ps_s.tile([P, P], f32, tag="mm_small")
            mm(out=d2p[:], lhsT=W["de_W2"][:], rhs=d1[:], start=True, stop=True)
            d2 = sbm.tile([P, P], bf16, tag="d2")
            nc.scalar.activation(out=d2[:], in_=d2p[:], func=Relu,
                                 bias=W["de_b2"][:, :1])
            dp = ps_s.tile([P, OD], f32, tag="mm_small")
            mm(out=dp[:], lhsT=d2[:], rhs=W["de_W3"][:], start=True, stop=False)
            mm(out=dp[:], lhsT=ones_row[:], rhs=W["de_b3"][:],
               start=False, stop=True)
            osb = sbm.tile([P, OD], f32, tag="osb")
            nc.vector.tensor_copy(osb[:], dp[:])
            nc.scalar.dma_start(out=out_d[ts(b, P), :], in_=osb[:])

    nc.finalize()
    return nc


def _ensure_ntff_hook():
    """Register the axon NTFF profiling hook if the image lacks
    antenv.axon_hooks (replicates trn_boot's ctypes wiring)."""
    import sys
    import types
    try:
        import antenv.axon_hooks  # noqa: F401
        return
    except ImportError:
        pass
    import contextlib
    import ctypes
    import antenv

    m = types.ModuleType("antenv.axon_hooks")
    state = {"hook": None, "tried": False}

    def set_axon_ntff_profile_hook(hook):
        state["hook"] = hook

    def _make_hook(so_path="/opt/axon/libaxon_pjrt.so"):
        lib = ctypes.CDLL(so_path)
        if not hasattr(lib, "axon_start_nrt_profile"):
            return None
        lib.axon_start_nrt_profile.argtypes = [
            ctypes.POINTER(ctypes.c_int64), ctypes.c_size_t]
        lib.axon_start_nrt_profile.restype = ctypes.c_int64
        lib.axon_stop_nrt_profile.argtypes = [ctypes.c_char_p]
        lib.axon_stop_nrt_profile.restype = ctypes.c_int64

        @contextlib.contextmanager
        def _hook(output_dir, device_ids):
            import jax
            jax.devices()
            if device_ids:
                ids = (ctypes.c_int64 * len(device_ids))(*device_ids)
                rc = lib.axon_start_nrt_profile(ids, len(device_ids))
            else:
                rc = lib.axon_start_nrt_profile(None, 0)
            if rc != 0:
                raise RuntimeError(f"axon_start_nrt_profile rc={rc}")
            try:
                yield
            finally:
                n = lib.axon_stop_nrt_profile(str(output_dir).encode())
                print(f"ntff profile: {n} file(s) written to {output_dir}")

        return _hook

    def get_axon_ntff_profile_hook():
        if state["hook"] is None and not state["tried"]:
            state["tried"] = True
            try:
                state["hook"] = _make_hook()
            except OSError:
                state["hook"] = None
        return state["hook"]

    m.set_axon_ntff_profile_hook = set_axon_ntff_profile_hook
    m.get_axon_ntff_profile_hook = get_axon_ntff_profile_hook
    sys.modules["antenv.axon_hooks"] = m
    antenv.axon_hooks = m


def kernel(**inputs):
    n_cores = 8
    params, in_maps = prep_host(inputs, n_cores)
    nc = build_program(params, debug=False)

    from concourse.bass_utils import run_bass_kernel_spmd
    import time
    trace = bool(int(os.environ.get("KERNEL_TRACE", "0")))
    if trace:
        try:
            _ensure_ntff_hook()
        except Exception:
            pass
    t0 = time.time()
    try:
        res = run_bass_kernel_spmd(nc, in_maps, list(range(n_cores)),
                                   trace=trace)
    except ModuleNotFoundError:
        res = run_bass_kernel_spmd(nc, in_maps, list(range(n_cores)),
                                   trace=False)
    LAST["wall_s"] = time.time() - t0
    LAST["exec_time_ns"] = getattr(res, "exec_time_ns", None)
    LAST["profile_json"] = getattr(res, "profile_json", None)
    LAST["params"] = params
    out = np.concatenate([r["out"] for r in res.results], axis=0)
    return np.ascontiguousarray(out[:params["N"]].astype(np.float32))
